# revision 1
# baseline (speedup 1.0000x reference)
"""AtomwiseLinear 3-expert MoE routing kernel for 8 TRN2 NeuronCores.

Strategy (data-parallel over atoms, per sharding hint):
  - Each core gets 125,000 atoms, padded to 126,720 = 33 chunks x 3840 atoms.
  - Per chunk: SWDGE DMA loads x (f32 in HBM) casting to bf16 in SBUF with a
    p-major layout (partition p holds G=30 consecutive atom rows).
  - PE transposes each [128 atom x 128 feat] square (bf16, 1 cyc/row) into
    PSUM, DVE copies batches of 5 squares back to SBUF, then PE computes
    out[a, 96] = xT.T @ Wcat for all 3 experts in one matmul (bf16, f32 accum).
  - Expert selection: out = P0; copy_predicated(out, ids, P1);
    copy_predicated(out, max(ids-1,0), P2) — masks broadcast along the
    32-wide output via zero-stride APs. Output DMA'd back as f32.
HBM traffic stays the honest f32 644 MB total; compute runs bf16.
"""

import sys

sys.path.insert(0, "/opt/trn_rl_repo")

import numpy as np
import ml_dtypes

import concourse.mybir as mybir
import concourse.bacc as bacc
import concourse.tile as tile
from concourse.bass_utils import run_bass_kernel_spmd

N_CORES = 8
N_ATOMS = 1_000_000
F_IN = 128
F_OUT = 32
P = 128
G = 30                     # atom-squares per chunk
HALF = 15                  # squares per half-chunk (one PSUM py tile)
CHUNK = P * G              # 3840 atoms
NCHUNKS = 33
NPAD = CHUNK * NCHUNKS     # 126720 >= 125000
NCORE = N_ATOMS // N_CORES # 125000

bf16 = mybir.dt.bfloat16
f32 = mybir.dt.float32
i32 = mybir.dt.int32

_NC_CACHE = {}


def build_bass():
    if "nc" in _NC_CACHE:
        return _NC_CACHE["nc"]
    nc = bacc.Bacc("TRN2", target_bir_lowering=False, debug=False,
                   num_devices=N_CORES)
    x_d = nc.dram_tensor("x", (NPAD, F_IN), f32, kind="ExternalInput")
    ids_d = nc.dram_tensor("expert_ids", (NPAD,), i32, kind="ExternalInput")
    w_d = nc.dram_tensor("wcat", (F_IN, 3 * F_OUT), f32, kind="ExternalInput")
    out_d = nc.dram_tensor("out", (NPAD, F_OUT), f32, kind="ExternalOutput")
    ident = nc.inline_tensor(np.eye(P, dtype=ml_dtypes.bfloat16), name="ident")

    with tile.TileContext(nc) as tc:
        with (
            tc.tile_pool(name="const", bufs=1) as cpool,
            tc.tile_pool(name="xin", bufs=3) as xpool,
            tc.tile_pool(name="xt", bufs=4) as xtpool,
            tc.tile_pool(name="outp", bufs=2) as opool,
            tc.tile_pool(name="pT", bufs=2, space="PSUM") as ptpool,
            tc.tile_pool(name="py", bufs=2, space="PSUM") as pypool,
        ):
            ident_sb = cpool.tile([P, P], bf16)
            nc.sync.dma_start(ident_sb[:], ident.ap())
            w_sb = cpool.tile([F_IN, 3 * F_OUT], bf16)
            nc.gpsimd.dma_start(w_sb[:], w_d.ap())  # f32 -> bf16 cast
            ids_sb = cpool.tile([P, NCHUNKS, G], i32)
            ids_ap = ids_d.ap().rearrange("(c p g) -> p c g",
                                          c=NCHUNKS, p=P, g=G)
            nc.sync.dma_start(ids_sb[:], ids_ap)
            m2_sb = cpool.tile([P, NCHUNKS, G], i32)
            nc.vector.tensor_scalar(
                m2_sb[:], ids_sb[:], 1, 0,
                op0=mybir.AluOpType.subtract, op1=mybir.AluOpType.max,
            )

            for c in range(NCHUNKS):
                xc = xpool.tile([P, G * F_IN], bf16)
                x_ap = x_d.ap()[c * CHUNK:(c + 1) * CHUNK, :].rearrange(
                    "(p g) f -> p (g f)", p=P)
                nc.gpsimd.dma_start(xc[:], x_ap)  # f32 -> bf16 cast
                ob = opool.tile([P, G * F_OUT], f32)
                for h in range(2):
                    py = pypool.tile([P, 3, 512], f32)  # 3 PSUM banks
                    for grp in range(3):  # 3 groups of 5 squares
                        pt = ptpool.tile([P, 5 * P], bf16)
                        for k in range(5):
                            g = h * HALF + grp * 5 + k
                            nc.tensor.transpose(
                                pt[:, k * P:(k + 1) * P],
                                xc[:, g * F_IN:(g + 1) * F_IN],
                                ident_sb[:])
                        xt = xtpool.tile([P, 5 * P], bf16)
                        nc.vector.tensor_copy(xt[:], pt[:])
                        for k in range(5):
                            b, s = divmod(grp * 5 + k, 5)
                            nc.tensor.matmul(
                                py[:, b, s * 96:s * 96 + 96],
                                xt[:, k * P:(k + 1) * P],
                                w_sb[:], start=True, stop=True)
                    # expert-select for this half chunk (15 squares)
                    pv = py[:, :, 0:480].rearrange("p b (s j) -> p b s j", j=96)
                    ov = ob[:, h * HALF * F_OUT:(h + 1) * HALF * F_OUT].rearrange(
                        "p (b s j) -> p b s j", b=3, s=5, j=F_OUT)
                    i0 = h * HALF
                    idv = ids_sb[:, c, i0:i0 + HALF].rearrange(
                        "p (b s) -> p b s", b=3)[:, :, :, None].broadcast_to(
                        [P, 3, 5, F_OUT])
                    m2v = m2_sb[:, c, i0:i0 + HALF].rearrange(
                        "p (b s) -> p b s", b=3)[:, :, :, None].broadcast_to(
                        [P, 3, 5, F_OUT])
                    nc.vector.tensor_copy(ov, pv[:, :, :, 0:F_OUT])
                    nc.vector.copy_predicated(ov, idv,
                                              pv[:, :, :, F_OUT:2 * F_OUT])
                    nc.vector.copy_predicated(ov, m2v,
                                              pv[:, :, :, 2 * F_OUT:3 * F_OUT])
                o_ap = out_d.ap()[c * CHUNK:(c + 1) * CHUNK, :].rearrange(
                    "(p g) f -> p (g f)", p=P)
                nc.sync.dma_start(o_ap, ob[:])
    nc.compile()
    _NC_CACHE["nc"] = nc
    return nc


def make_in_maps(x, W1, W2, W3, expert_ids):
    x = np.ascontiguousarray(np.asarray(x, dtype=np.float32))
    ids = np.ascontiguousarray(np.asarray(expert_ids, dtype=np.int32))
    wcat = np.concatenate(
        [np.asarray(W1, np.float32), np.asarray(W2, np.float32),
         np.asarray(W3, np.float32)], axis=1)
    wcat = np.ascontiguousarray(wcat)
    in_maps = []
    for c in range(N_CORES):
        xs = np.zeros((NPAD, F_IN), np.float32)
        xs[:NCORE] = x[c * NCORE:(c + 1) * NCORE]
        isd = np.zeros((NPAD,), np.int32)
        isd[:NCORE] = ids[c * NCORE:(c + 1) * NCORE]
        in_maps.append({"x": xs, "expert_ids": isd, "wcat": wcat})
    return in_maps


def kernel(x, W1, W2, W3, expert_ids):
    nc = build_bass()
    in_maps = make_in_maps(x, W1, W2, W3, expert_ids)
    res = run_bass_kernel_spmd(nc, in_maps, core_ids=list(range(N_CORES)))
    out = np.concatenate(
        [res.results[c]["out"][:NCORE] for c in range(N_CORES)], axis=0)
    return np.ascontiguousarray(out.astype(np.float32, copy=False))


if __name__ == "__main__":
    rng = np.random.default_rng(0)
    x = rng.standard_normal((N_ATOMS, F_IN)).astype(np.float32)
    ids = rng.integers(0, 3, N_ATOMS).astype(np.int32)
    sc = 1.0 / np.sqrt(F_IN)
    W1, W2, W3 = (rng.standard_normal((F_IN, F_OUT)).astype(np.float32) * sc
                  for _ in range(3))
    out = kernel(x, W1, W2, W3, ids)
    exact = np.stack([x @ W1, x @ W2, x @ W3])[ids, np.arange(N_ATOMS)]
    rel = np.linalg.norm(out - exact) / np.linalg.norm(exact)
    print("rel err vs exact f32:", rel)


# revision 4
# speedup vs baseline: 224.2166x; 224.2166x over previous
"""AtomwiseLinear 3-expert MoE routing kernel for 8 TRN2 NeuronCores.

Strategy (data-parallel over atoms, per sharding hint):
  - Each core gets 125,000 atoms, padded to 126,720 = 33 chunks x 3840 atoms.
  - Per chunk: SWDGE DMA loads x (f32 in HBM) casting to bf16 in SBUF with a
    p-major layout (partition p holds G=30 consecutive atom rows).
  - PE transposes each [128 atom x 128 feat] square (bf16, 1 cyc/row) into
    PSUM, DVE copies batches of 5 squares back to SBUF, then PE computes
    out[a, 96] = xT.T @ Wcat for all 3 experts in one matmul (bf16, f32 accum).
  - Expert selection: out = P0; copy_predicated(out, ids, P1);
    copy_predicated(out, max(ids-1,0), P2) — masks broadcast along the
    32-wide output via zero-stride APs. Output DMA'd back as f32.
HBM traffic stays the honest f32 644 MB total; compute runs bf16.
"""

import sys

sys.path.insert(0, "/opt/trn_rl_repo")

import numpy as np
import ml_dtypes

import concourse.mybir as mybir
import concourse.bacc as bacc
import concourse.tile as tile
from concourse.bass_utils import run_bass_kernel_spmd

N_CORES = 8
N_ATOMS = 1_000_000
F_IN = 128
F_OUT = 32
P = 128
G = 30                     # atom-squares per chunk
HALF = 15                  # squares per half-chunk (one PSUM py tile)
CHUNK = P * G              # 3840 atoms
NCHUNKS = 33
NPAD = CHUNK * NCHUNKS     # 126720 >= 125000
NCORE = N_ATOMS // N_CORES # 125000

bf16 = mybir.dt.bfloat16
f32 = mybir.dt.float32
i32 = mybir.dt.int32

_NC_CACHE = {}


def build_bass(repeat=0):
    """repeat=0: production kernel. repeat=R>0: wraps the whole body in an
    on-device For_i loop running it R times (timing-only variant — lets a
    differential wall-clock measurement cancel host/RPC dispatch overhead)."""
    key = ("nc", repeat)
    if key in _NC_CACHE:
        return _NC_CACHE[key]
    nc = bacc.Bacc("TRN2", target_bir_lowering=False, debug=False,
                   num_devices=N_CORES)
    x_d = nc.dram_tensor("x", (NPAD, F_IN), f32, kind="ExternalInput")
    ids_d = nc.dram_tensor("expert_ids", (NPAD,), i32, kind="ExternalInput")
    w_d = nc.dram_tensor("wcat", (F_IN, 3 * F_OUT), f32, kind="ExternalInput")
    out_d = nc.dram_tensor("out", (NPAD, F_OUT), f32, kind="ExternalOutput")
    ident = nc.inline_tensor(np.eye(P, dtype=ml_dtypes.bfloat16), name="ident")

    with tile.TileContext(nc) as tc:
        with (
            tc.tile_pool(name="const", bufs=1) as cpool,
            tc.tile_pool(name="xin", bufs=3) as xpool,
            tc.tile_pool(name="xt", bufs=4) as xtpool,
            tc.tile_pool(name="outp", bufs=2) as opool,
            tc.tile_pool(name="pT", bufs=2, space="PSUM") as ptpool,
            tc.tile_pool(name="py", bufs=2, space="PSUM") as pypool,
        ):
            ident_sb = cpool.tile([P, P], bf16)
            nc.sync.dma_start(ident_sb[:], ident.ap())
            w_sb = cpool.tile([F_IN, 3 * F_OUT], bf16)
            nc.gpsimd.dma_start(w_sb[:], w_d.ap())  # f32 -> bf16 cast
            ids_sb = cpool.tile([P, NCHUNKS, G], i32)
            ids_ap = ids_d.ap().rearrange("(c p g) -> p c g",
                                          c=NCHUNKS, p=P, g=G)
            nc.sync.dma_start(ids_sb[:], ids_ap)
            m2_sb = cpool.tile([P, NCHUNKS, G], i32)
            nc.vector.tensor_scalar(
                m2_sb[:], ids_sb[:], 1, 0,
                op0=mybir.AluOpType.subtract, op1=mybir.AluOpType.max,
            )

            import contextlib
            loop_ctx = (tc.For_i(0, repeat, 1) if repeat
                        else contextlib.nullcontext())
            with loop_ctx:
                _body(nc, tc, x_d, out_d, xpool, xtpool, opool, ptpool, pypool,
                      ident_sb, w_sb, ids_sb, m2_sb)
    nc.compile()
    _NC_CACHE[key] = nc
    return nc


def _body(nc, tc, x_d, out_d, xpool, xtpool, opool, ptpool, pypool,
          ident_sb, w_sb, ids_sb, m2_sb):
    if True:
        if True:
            for c in range(NCHUNKS):
                xc = xpool.tile([P, G * F_IN], bf16)
                x_ap = x_d.ap()[c * CHUNK:(c + 1) * CHUNK, :].rearrange(
                    "(p g) f -> p (g f)", p=P)
                nc.gpsimd.dma_start(xc[:], x_ap)  # f32 -> bf16 cast
                ob = opool.tile([P, G * F_OUT], f32)
                for h in range(2):
                    py = pypool.tile([P, 3, 512], f32)  # 3 PSUM banks
                    for grp in range(3):  # 3 groups of 5 squares
                        pt = ptpool.tile([P, 5 * P], bf16)
                        for k in range(5):
                            g = h * HALF + grp * 5 + k
                            nc.tensor.transpose(
                                pt[:, k * P:(k + 1) * P],
                                xc[:, g * F_IN:(g + 1) * F_IN],
                                ident_sb[:])
                        xt = xtpool.tile([P, 5 * P], bf16)
                        nc.vector.tensor_copy(xt[:], pt[:])
                        for k in range(5):
                            b, s = divmod(grp * 5 + k, 5)
                            nc.tensor.matmul(
                                py[:, b, s * 96:s * 96 + 96],
                                xt[:, k * P:(k + 1) * P],
                                w_sb[:], start=True, stop=True)
                    # expert-select for this half chunk (15 squares)
                    pv = py[:, :, 0:480].rearrange("p b (s j) -> p b s j", j=96)
                    ov = ob[:, h * HALF * F_OUT:(h + 1) * HALF * F_OUT].rearrange(
                        "p (b s j) -> p b s j", b=3, s=5, j=F_OUT)
                    i0 = h * HALF
                    idv = ids_sb[:, c, i0:i0 + HALF].rearrange(
                        "p (b s) -> p b s", b=3)[:, :, :, None].broadcast_to(
                        [P, 3, 5, F_OUT])
                    m2v = m2_sb[:, c, i0:i0 + HALF].rearrange(
                        "p (b s) -> p b s", b=3)[:, :, :, None].broadcast_to(
                        [P, 3, 5, F_OUT])
                    nc.vector.tensor_copy(ov, pv[:, :, :, 0:F_OUT])
                    nc.vector.copy_predicated(ov, idv,
                                              pv[:, :, :, F_OUT:2 * F_OUT])
                    nc.vector.copy_predicated(ov, m2v,
                                              pv[:, :, :, 2 * F_OUT:3 * F_OUT])
                o_ap = out_d.ap()[c * CHUNK:(c + 1) * CHUNK, :].rearrange(
                    "(p g) f -> p (g f)", p=P)
                nc.sync.dma_start(o_ap, ob[:])


def make_in_maps(x, W1, W2, W3, expert_ids):
    x = np.ascontiguousarray(np.asarray(x, dtype=np.float32))
    ids = np.ascontiguousarray(np.asarray(expert_ids, dtype=np.int32))
    wcat = np.concatenate(
        [np.asarray(W1, np.float32), np.asarray(W2, np.float32),
         np.asarray(W3, np.float32)], axis=1)
    wcat = np.ascontiguousarray(wcat)
    in_maps = []
    for c in range(N_CORES):
        xs = np.zeros((NPAD, F_IN), np.float32)
        xs[:NCORE] = x[c * NCORE:(c + 1) * NCORE]
        isd = np.zeros((NPAD,), np.int32)
        isd[:NCORE] = ids[c * NCORE:(c + 1) * NCORE]
        in_maps.append({"x": xs, "expert_ids": isd, "wcat": wcat})
    return in_maps


def kernel(x, W1, W2, W3, expert_ids):
    nc = build_bass()
    in_maps = make_in_maps(x, W1, W2, W3, expert_ids)
    res = run_bass_kernel_spmd(nc, in_maps, core_ids=list(range(N_CORES)))
    out = np.concatenate(
        [res.results[c]["out"][:NCORE] for c in range(N_CORES)], axis=0)
    return np.ascontiguousarray(out.astype(np.float32, copy=False))


if __name__ == "__main__":
    rng = np.random.default_rng(0)
    x = rng.standard_normal((N_ATOMS, F_IN)).astype(np.float32)
    ids = rng.integers(0, 3, N_ATOMS).astype(np.int32)
    sc = 1.0 / np.sqrt(F_IN)
    W1, W2, W3 = (rng.standard_normal((F_IN, F_OUT)).astype(np.float32) * sc
                  for _ in range(3))
    out = kernel(x, W1, W2, W3, ids)
    exact = np.stack([x @ W1, x @ W2, x @ W3])[ids, np.arange(N_ATOMS)]
    rel = np.linalg.norm(out - exact) / np.linalg.norm(exact)
    print("rel err vs exact f32:", rel)
